# revision 1
# baseline (speedup 1.0000x reference)
"""Trainium2 Bass kernel for nn_BasicRecurrentEntityEncoder (v2).

Math (per paragraph b, per step t, state h [K, D]):
    g   = sigmoid(s . (h + keys))            per entity gate
    ht  = relu(h U + keys V + s W)
    upd = h + g * ht
    h'  = upd / ||upd||_2   where active (mask), else h unchanged

v2 design (8 cores, pure data parallel, 32 paragraphs/core):
  * rows r = (paragraph, entity) = 2048 rows/core, 16 tiles of 128,
    split into 2 independent groups of 8 tiles pipelined across engines.
  * state u (fp16, UNNORMALIZED) + transposed shadow uT; per-row scalar
    dl = rsqrt(||u||^2) applied lazily:  h = dl * u.
  * recurrence rewritten so the critical path is
      PE(ps1 = uU, gate = s.u) -> gate scalars -> m = relu(gd*ps1 + g*kvsw)
      -> u' = dl*u + m -> DMA-transpose(u')
    with n2/dl computed OFF the chain (only needed next step), and
    kv + sw_t pre-merged on GPSIMD into double-buffered kvsw slots.
  * pad handling: host bakes SK[r, t] = s.keys - 60*(step padded), so
    gamma = sigmoid(dl*(s.u) + SK) -> ~0 on padded steps; dl recurrence
    self-normalizes (u -> u/||u||) so no pad mask is needed anywhere.
  * dl_0 = 0 makes step 0 exact (h_0 = 0).
"""

import os
import sys

sys.path.insert(0, "/opt/trn_rl_repo")

import numpy as np
import ml_dtypes
from contextlib import ExitStack

import concourse.bass as bass
import concourse.bacc as bacc
import concourse.mybir as mybir
from concourse.tile import TileContext

F16 = mybir.dt.float16
F32 = mybir.dt.float32
AF = mybir.ActivationFunctionType
ALU = mybir.AluOpType

B, S, K, D = 256, 256, 64, 128
NCORES = 8
BL = B // NCORES  # 32 local paragraphs
NT = 16  # row tiles per core (2048 rows / 128)
TPG = 8  # tiles per group
PAD_NEG = 60.0


# ------------------------------------------------------------------ custom op
def get_relu_comb_op():
    """out = relu(in0*s0 + in1*s1)   (relu(gamdl*ps1 + gam*kvsw))."""
    from concourse import dve_ops as dv
    from concourse.dve_spec import Spec, Src0, Src1, C0, C1, relu, lower, _has_src1
    from concourse.dve_uop import DveOpSpec

    name = "RELU_AXPBY_ANT"
    for o in dv.OPS:
        if o.name == name:
            return o

    def _ref(in0, in1, s0, s1, imm2):
        x = in0.astype(np.float32) * s0 + in1.astype(np.float32) * s1
        x = np.nan_to_num(x, nan=0.0, posinf=np.inf, neginf=-np.inf)
        return np.maximum(x, 0.0)

    spec = Spec(body=relu(Src0 * C0 + Src1 * C1), reference=_ref)
    row = max(dv._SUB_OPCODE_FOR_NAME.values()) + 1
    assert row < 0x20, "no free custom-DVE opcode rows"
    dv._SUB_OPCODE_FOR_NAME[name] = row
    shas = {}
    for ver in ("v3", "v4"):
        try:
            uops = lower(spec, ver=ver)
            shas[ver] = DveOpSpec(
                name=name, opcode=row, uops=uops, rd1_en=_has_src1(spec)
            ).sha(ver)
        except Exception:
            pass
    assert "v3" in shas, "custom op failed to lower for TRN2"
    op = dv.DveOp(name, spec, subdim=False, uops_sha=shas)
    dv.OPS.append(op)
    dv.CUSTOM_DVE_SPECS[name] = spec
    return op


# ------------------------------------------------------------------ program
def build_program(T, sim=False, repeat=1):
    """Emit the full per-core Bass program. Returns nc.

    repeat > 1 re-emits the time loop (timing runs only: the wall-clock
    slope vs repeat isolates steady-state loop time from fixed overhead).
    """
    op_m = get_relu_comb_op()
    nc = bacc.Bacc("TRN2", target_bir_lowering=False)

    # ---- I/O
    NF16 = NT * 128 + T * BL + 4 * 128  # keysT | sT | U | V | W | ident
    blob_in = nc.dram_tensor("blob16", [128, NF16], F16, kind="ExternalInput")
    sk_in = nc.dram_tensor("skblob", [128, T, NT], F32, kind="ExternalInput")
    hfin_out = nc.dram_tensor("hfin", [128, NT, 128], F32, kind="ExternalOutput")
    sw_dram = nc.dram_tensor("sw_stage", [T, BL, 128], F16, kind="Internal")

    TC = (T * BL + 127) // 128  # 128-col chunks of the (t, j) axis

    with ExitStack() as ctx:
        tc = ctx.enter_context(TileContext(nc))
        ec = ctx.enter_context

        # ---- persistent SBUF
        blob_sb = ec(nc.sbuf_tensor("blob_sb", [128, NF16], F16))
        o = 0
        keysT_sb = blob_sb[:, o : o + NT * 128].rearrange(
            "p (i e) -> p i e", i=NT
        ); o += NT * 128
        sT_sb = blob_sb[:, o : o + T * BL].rearrange(
            "p (t j) -> p t j", t=T
        ); o += T * BL
        U_sb = blob_sb[:, o : o + 128]; o += 128
        V_sb = blob_sb[:, o : o + 128]; o += 128
        W_sb = blob_sb[:, o : o + 128]; o += 128
        I_sb = blob_sb[:, o : o + 128]; o += 128

        SK_sb = ec(nc.sbuf_tensor("SK_sb", [128, T, NT], F32))
        kv_sb = ec(nc.sbuf_tensor("kv_sb", [128, NT, 128], F16))
        u_sb = ec(nc.sbuf_tensor("u_sb", [128, NT, 128], F16))
        uT_sb = ec(nc.sbuf_tensor("uT_sb", [128, NT, 128], F16))
        m_sb = ec(nc.sbuf_tensor("m_sb", [128, NT, 128], F16))
        sqs_sb = ec(nc.sbuf_tensor("sqs_sb", [128, NT, 128], F16))
        swrep_sb = ec(nc.sbuf_tensor("swrep_sb", [128, 2, NT, 128], F16))
        kvsw_sb = ec(nc.sbuf_tensor("kvsw_sb", [128, 2, NT, 128], F16))
        swT_sb = ec(nc.sbuf_tensor("swT_sb", [128, TC * 128], F16))
        swch_sb = ec(nc.sbuf_tensor("swch_sb", [128, TC, 128], F16))
        hfin_sb = ec(nc.sbuf_tensor("hfin_sb", [128, NT, 128], F32))
        # small per-row scalars
        dl_sb = ec(nc.sbuf_tensor("dl_sb", [128, NT], F32))
        gtmp_sb = ec(nc.sbuf_tensor("gtmp_sb", [128, NT], F32))
        glog_sb = ec(nc.sbuf_tensor("glog_sb", [128, NT], F32))
        ex_sb = ec(nc.sbuf_tensor("ex_sb", [128, NT], F32))
        den_sb = ec(nc.sbuf_tensor("den_sb", [128, NT], F32))
        gam_sb = ec(nc.sbuf_tensor("gam_sb", [128, NT], F32))
        gamdl_sb = ec(nc.sbuf_tensor("gamdl_sb", [128, NT], F32))
        n2_sb = ec(nc.sbuf_tensor("n2_sb", [128, NT], F32))
        lgn_sb = ec(nc.sbuf_tensor("lgn_sb", [128, NT], F32))
        # psum: ps1 4 banks + 2 gate banks + aux 2 banks = 8
        ps1 = ec(nc.psum_tensor("ps1", [128, NT * 128], F32)).rearrange(
            "p (i e) -> p i e", i=NT
        )
        ps_gA = ec(nc.psum_tensor("ps_gA", [128, TPG * 2], F32)).rearrange(
            "p (i j) -> p i j", j=2
        )
        ps_gB = ec(nc.psum_tensor("ps_gB", [128, TPG * 2], F32)).rearrange(
            "p (i j) -> p i j", j=2
        )
        ps_aux = ec(nc.psum_tensor("ps_aux", [128, 1024], F32))

        sync = nc.sync
        vec = nc.vector
        act = nc.scalar
        gps = nc.gpsimd
        pe = nc.tensor

        lo, hi = slice(0, 64), slice(64, 128)

        # ================= setup =================
        sync.dma_start(blob_sb[:], blob_in[:], max_dma_last_dim=65024)
        sync.dma_start(SK_sb[:], sk_in[:], max_dma_last_dim=65024)

        vec.memset(u_sb[:], 0)
        vec.memset(uT_sb[:], 0)
        vec.memset(dl_sb[:], 0.0)  # dl_0 = 0: step 0 uses h = 0 exactly

        # kv = keys @ V   (natural tiles; ps_aux bank 0 in rounds of 4)
        for c in range(4):
            for q in range(4):
                i = 4 * c + q
                pe.matmul(
                    ps_aux[:, q * 128 : (q + 1) * 128],
                    lhsT=keysT_sb[:, i, :],
                    rhs=V_sb,
                    start=(q == 0),
                    stop=(q == 3),
                )
            vec.tensor_copy(
                kv_sb[:, 4 * c : 4 * (c + 1), :].rearrange("p i e -> p (i e)"),
                ps_aux[:, 0:512],
            )

        # sW staging: swT = W^T @ s  -> chunked transpose -> DRAM [T, BL, 128]
        vec.memset(swT_sb[:], 0)
        for c in range(TC):
            cpos = c * 128
            n = min(128, T * BL - cpos)
            cols = sT_sb.rearrange("d t j -> d (t j)")[:, cpos : cpos + n]
            bank = 512 + (c % 2) * 128  # ping-pong in aux bank 1
            pe.matmul(
                ps_aux[:, bank : bank + n],
                lhsT=W_sb,
                rhs=cols,
                start=True,
                stop=True,
            )
            vec.tensor_copy(swT_sb[:, cpos : cpos + n], ps_aux[:, bank : bank + n])
        sync.dma_start_transpose(swch_sb[:], swT_sb[:])
        # swch[32*(t%4)+j, t//4, e] = sW[t, j, e]  ->  sw_dram[t, j, e]
        for t4 in range(4):
            n_c = (T - t4 + 3) // 4  # chunks c with t = 4c + t4 < T
            if n_c <= 0:
                continue
            src = swch_sb[32 * t4 : 32 * (t4 + 1), 0:n_c, :]
            hi_t = t4 + 4 * (n_c - 1) + 1
            dst = sw_dram[t4:hi_t:4, :, :].rearrange("c j e -> j c e")
            sync.dma_start(dst, src)

        def prefetch_sw(t):
            slot = t % 2
            for a in range(2):
                va = sw_dram[t, a : BL : 2, :].rearrange("i (q e) -> q i e", q=1)
                vab = va.broadcast_to([64, NT, 128])
                sync.dma_start(swrep_sb[64 * a : 64 * (a + 1), slot, :, :], vab)

        def prep_kvsw(t):
            slot = t % 2
            gps.tensor_tensor(
                kvsw_sb[:, slot, :, :],
                kv_sb[:],
                swrep_sb[:, slot, :, :],
                op=ALU.add,
            )

        # ================= time loop =================
        def emit_time_loop():
            for t in range(T):
                slot = t % 2
                for g in range(2):
                    t0 = TPG * g
                    gs = slice(t0, t0 + TPG)
                    ps_g = ps_gA if g == 0 else ps_gB

                    # ---- PE: ps1_i = u U ; gate = u . s ; += kvsw
                    for q in range(TPG):
                        i = t0 + q
                        pe.matmul(
                            ps1[:, i, :],
                            lhsT=uT_sb[:, i, :],
                            rhs=U_sb,
                            start=(i % 4 == 0),
                            stop=(i % 4 == 3),
                        )
                        pe.matmul(
                            ps_g[:, q, :],
                            lhsT=uT_sb[:, i, :],
                            rhs=sT_sb[:, t, 2 * i : 2 * i + 2],
                            start=(q == 0),
                            stop=(q == TPG - 1),
                        )

                    # ---- gate: gam = sigmoid(dl*(s.u) + SK)
                    vec.tensor_tensor(
                        gtmp_sb[lo, gs], ps_g[lo, :, 0], dl_sb[lo, gs], op=ALU.mult
                    )
                    vec.tensor_tensor(
                        gtmp_sb[hi, gs], ps_g[hi, :, 1], dl_sb[hi, gs], op=ALU.mult
                    )
                    vec.tensor_tensor(
                        glog_sb[:, gs], gtmp_sb[:, gs], SK_sb[:, t, gs], op=ALU.add
                    )
                    act.activation(ex_sb[:, gs], glog_sb[:, gs], AF.Exp, scale=-1.0)
                    vec.tensor_scalar_add(den_sb[:, gs], ex_sb[:, gs], 1.0)
                    vec.reciprocal(gam_sb[:, gs], den_sb[:, gs])
                    vec.tensor_tensor(
                        gamdl_sb[:, gs], gam_sb[:, gs], dl_sb[:, gs], op=ALU.mult
                    )

                    # ---- m = relu(gamdl*ps1 + gam*kvsw) ; u' = dl*u + m
                    for q in range(TPG):
                        i = t0 + q
                        vec._custom_dve(
                            op_m,
                            out=m_sb[:, i, :],
                            in0=ps1[:, i, :],
                            in1=kvsw_sb[:, slot, i, :],
                            s0=gamdl_sb[:, i : i + 1],
                            s1=gam_sb[:, i : i + 1],
                        )
                    for q in range(TPG):
                        i = t0 + q
                        vec.scalar_tensor_tensor(
                            u_sb[:, i, :],
                            in0=u_sb[:, i, :],
                            scalar=dl_sb[:, i : i + 1],
                            in1=m_sb[:, i, :],
                            op0=ALU.mult,
                            op1=ALU.add,
                        )

                    # ---- transposed shadow for next step's PE
                    sync.dma_start_transpose(
                        uT_sb[:, gs, :],
                        u_sb[:, gs, :].rearrange("p i e -> p (i e)"),
                    )

                    # ---- off-chain: n2 -> dl' = rsqrt(n2) (used next step)
                    for q in range(TPG):
                        i = t0 + q
                        act.activation(
                            sqs_sb[:, i, :],
                            u_sb[:, i, :],
                            AF.Square,
                            accum_out=n2_sb[:, i : i + 1],
                        )
                    vec.tensor_scalar_max(n2_sb[:, gs], n2_sb[:, gs], 1e-12)
                    act.activation(lgn_sb[:, gs], n2_sb[:, gs], AF.Ln)
                    act.activation(dl_sb[:, gs], lgn_sb[:, gs], AF.Exp, scale=-0.5)

                if t + 2 < T:
                    prefetch_sw(t + 2)
                    prep_kvsw(t + 2)


        for rep in range(repeat):
            prefetch_sw(0)
            prep_kvsw(0)
            if T > 1:
                prefetch_sw(1)
                prep_kvsw(1)
            emit_time_loop()
        # ================= output =================
        for i in range(NT):
            act.activation(
                hfin_sb[:, i, :],
                u_sb[:, i, :],
                AF.Copy,
                scale=dl_sb[:, i : i + 1],
            )
        sync.dma_start(hfin_out[:], hfin_sb[:])

    nc.compile()
    return nc


# ------------------------------------------------------------------ host prep
def prepare_inputs(encoded_sents, mask, keys, U, V, W):
    """Build per-core input maps + metadata. Returns (T, in_maps)."""
    es = np.asarray(encoded_sents, dtype=np.float32)
    mk = np.asarray(mask)
    ks = np.asarray(keys, dtype=np.float32)

    nb = mk.sum(axis=1).astype(np.int64)  # active counts per paragraph
    T = int(nb.max()) if nb.max() > 0 else 1

    f16 = np.float16
    U_h = np.asarray(U, dtype=np.float32).astype(f16)
    V_h = np.asarray(V, dtype=np.float32).astype(f16)
    W_h = np.asarray(W, dtype=np.float32).astype(f16)
    ident = np.eye(128, dtype=np.float32).astype(f16)

    q = np.arange(128)
    i_idx = np.arange(NT)
    b_loc = 2 * i_idx[None, :] + (q[:, None] >= 64)  # [128, NT]
    k_of_q = q % 64

    in_maps = []
    for c in range(NCORES):
        bs = np.arange(BL) + BL * c  # global paragraph ids
        s_comp = np.zeros((BL, T, D), np.float32)
        padm = np.zeros((BL, T), np.float32)
        for j, b in enumerate(bs):
            idx = np.nonzero(mk[b])[0]
            n = len(idx)
            if n:
                s_comp[j, :n] = es[b, idx]
                padm[j, :n] = 1.0

        # sT[d, t, j]
        sT = np.ascontiguousarray(s_comp.transpose(2, 1, 0)).astype(f16)

        # keysT[d, i, p] = keys[b(i,p), k(p), d]
        kk = ks[bs]  # [BL, K, D]
        keysT = np.ascontiguousarray(
            kk[b_loc, k_of_q[:, None], :].transpose(2, 1, 0)
        ).astype(f16)  # [D, NT, 128]

        # SK[p, t, i] = s_comp[b(p,i), t, :] . keys[b(p,i), k(p), :]
        #              - 60 where padded
        sk_full = np.einsum("jtd,jkd->jtk", s_comp, kk)  # [BL, T, K]
        SKc = sk_full[
            b_loc[:, None, :], np.arange(T)[None, :, None], k_of_q[:, None, None]
        ]  # [128, T, NT]
        padsel = padm[b_loc[:, None, :], np.arange(T)[None, :, None]]
        SKc = (SKc + (padsel - 1.0) * PAD_NEG).astype(np.float32)

        blob = np.concatenate(
            [
                keysT.reshape(D, NT * 128),
                sT.reshape(D, T * BL),
                U_h,
                V_h,
                W_h,
                ident,
            ],
            axis=1,
        ).astype(f16)
        in_maps.append(
            {
                "blob16": np.ascontiguousarray(blob),
                "skblob": np.ascontiguousarray(SKc),
            }
        )
    return T, in_maps


def gather_output(results):
    """results: list of dicts with 'hfin' [128, NT, 128] per core -> [B, K, D]."""
    out = np.zeros((B, K, D), np.float32)
    for c in range(NCORES):
        h = results[c]["hfin"]  # [128, NT, 128]
        for b_loc in range(BL):
            i, a = b_loc // 2, b_loc % 2
            out[BL * c + b_loc] = h[64 * a : 64 * a + 64, i, :]
    return out


# ------------------------------------------------------------------ entry
def kernel(encoded_sents, mask, keys, U, V, W):
    from concourse.bass_utils import run_bass_kernel_spmd

    T, in_maps = prepare_inputs(encoded_sents, mask, keys, U, V, W)
    nc = build_program(T)
    res = run_bass_kernel_spmd(nc, in_maps, core_ids=list(range(NCORES)))
    return gather_output(res.results)


# ------------------------------------------------------------------ sim check
def _sim_check(tsteps=6):
    """CoreSim single-core run on truncated data vs reference."""
    from concourse import bass_interp

    sys.path.insert(0, os.path.dirname(os.path.abspath(__file__)))
    import reference

    inputs = {k: np.asarray(v) for k, v in reference.setup_inputs().items()}
    mask = inputs["mask"].copy()
    for b in range(B):
        idx = np.nonzero(mask[b])[0]
        mask[b, idx[tsteps:]] = False
    inputs["mask"] = mask

    ref = np.asarray(
        reference.reference(
            inputs["encoded_sents"],
            mask,
            inputs["keys"],
            inputs["U"],
            inputs["V"],
            inputs["W"],
        )
    )

    T, in_maps = prepare_inputs(
        inputs["encoded_sents"], mask, inputs["keys"],
        inputs["U"], inputs["V"], inputs["W"],
    )
    print(f"sim T={T}")
    nc = build_program(T, sim=True)
    core = 0
    sim = bass_interp.CoreSim(nc)
    for k, v in in_maps[core].items():
        sim.tensor(k)[:] = v
    sim.simulate()
    got = gather_output([{"hfin": np.array(sim.tensor("hfin"))}] * NCORES)

    g0 = got[:BL]
    r0 = ref[:BL]
    denom = np.abs(r0).max()
    err = np.abs(g0 - r0).max() / denom
    rel = np.linalg.norm(g0 - r0) / np.linalg.norm(r0)
    print(f"sim core0: absmax-rel {err:.3e}  l2-rel {rel:.3e}")
    return err


if __name__ == "__main__":
    _sim_check(int(sys.argv[1]) if len(sys.argv) > 1 else 6)

